# revision 1
# baseline (speedup 1.0000x reference)
"""Depth-modulated 3x3 conv (DepthConv2d) Trainium2 Bass kernel.

Math (per batch image):
  out[o, y, x] = bias[o] + sum_{c,k} w[o,c,k] * img[c, (y,x)+delta_k] * sim[k, (y,x)]
  sim[k, p]    = exp(-8.3 * |D_pad[p + delta_k] - D_pad[p + delta_0]|)
  (k = 3*i + j, i,j in 0..2; offsets relative to the padded top-left corner)

Strategy (8 NeuronCores, data-parallel over batch B=8, one image per core):
  - Padded image [64, 130, 130] in SBUF twice on the partition axis:
    partitions 0-63 hold P, partitions 64-127 hold P shifted +1 flat
    position.  A single [128, 512] view then covers two offsets at once
    (lower half offset s, upper half offset s+1).
  - Offsets are grouped into 4 "wide" K=128 groups + 1 narrow K=64 group:
      pair p in {0,1,2}: offsets (3p, 3p+1)   via view row y0+p, col 0
      merged single:     offsets (2, 5)       k2 from the lower copy
                                              (row y0, col 2), k5 from the
                                              upper copy (row y0+1, col 1)
      single:            offset 8             lower copy, row y0+2, col 2
  - sim computed in [y, x] layout on DVE/ACT, flattened to [9, 16384] via a
    DRAM round trip.  Per 512-pixel tile, sim rows are replicated across
    partitions with tiny K=9 fp32r matmuls (PE -> PSUM), the image views are
    modulated on DVE (tensor_tensor vs PSUM), and 5 accumulating fp32r
    matmuls contract (c, k) -> 64 output channels.
  - Bias is added on the ACT engine while copying PSUM->SBUF, then DMA out.
"""

import numpy as np

import concourse.bass as bass
import concourse.mybir as mybir
import concourse.tile as tile
from concourse.bass_utils import run_bass_kernel_spmd

F32 = mybir.dt.float32
F32R = mybir.dt.float32r
ALPHA = 8.3
N_CORES = 8

_WAIT_CAP = 1  # walrus engine-instruction sync-wait slot limit
_EV_CAP = 2  # InstEventSemaphore holds up to 2 waits


def _split_excess_waits(nc):
    """Move excess sync waits (>_WAIT_CAP) off engine instructions onto
    standalone InstEventSemaphore carriers inserted just before, on the same
    engine.  Tile's scheduler often leaves 2+ waits on one instruction,
    which walrus codegen rejects ("Too many sync wait commands")."""
    import bass_rust

    for bb in nc.main_func.blocks:
        out = []
        for ins in bb.instructions:
            si = ins.sync_info
            tname = type(ins).__name__
            if (
                si is not None
                and si.on_wait
                and len(si.on_wait) > _WAIT_CAP
                and tname != "InstEventSemaphore"
            ):
                waits = list(si.on_wait)
                keep = waits[-_WAIT_CAP:]
                excess = waits[:-_WAIT_CAP]
                while excess:
                    chunk, excess = excess[:_EV_CAP], excess[_EV_CAP:]
                    ev = bass_rust.InstEventSemaphore(
                        name=nc.get_next_instruction_name(), ins=[], outs=[]
                    )
                    ev.engine = ins.engine
                    ev.sync_info = bass_rust.SyncInfo(on_wait=chunk, on_update=[])
                    out.append(ev)
                si.on_wait = keep
            out.append(ins)
        bb.instructions[:] = out


def build_bass(loop_reps: int = 0, probe: str = ""):
    nc = bass.Bass()
    img_d = nc.dram_tensor("image", [64, 128, 128], F32, kind="ExternalInput")
    dep_d = nc.dram_tensor("depth", [128, 128], F32, kind="ExternalInput")
    wp_d = nc.dram_tensor("wp", [128, 4, 64], F32, kind="ExternalInput")
    ws8_d = nc.dram_tensor("ws8", [64, 64], F32, kind="ExternalInput")
    sp_d = nc.dram_tensor("sp", [9, 4, 128], F32, kind="ExternalInput")
    ss8_d = nc.dram_tensor("ss8", [9, 64], F32, kind="ExternalInput")
    bias_d = nc.dram_tensor("bias", [64, 1], F32, kind="ExternalInput")
    out_d = nc.dram_tensor("out", [64, 128, 128], F32, kind="ExternalOutput")

    with tile.TileContext(nc) as tc:
        with (
            tc.tile_pool(name="singles", bufs=1) as singles,
            tc.tile_pool(name="dram", bufs=1, space="DRAM") as drampool,
            tc.tile_pool(name="reps", bufs=5, space="PSUM") as reps,
            tc.tile_pool(name="outps", bufs=2, space="PSUM") as outps,
            tc.tile_pool(name="mods", bufs=6) as mods,
            tc.tile_pool(name="repsb", bufs=4) as repsb,
            tc.tile_pool(name="outsb", bufs=4) as outsb,
        ):
            # ---- padded image, two arrangements on the partition axis,
            # split into 4 overlapping row blocks so the first tiles can
            # start before the whole image is resident.  Block b holds
            # padded rows [32b, 32b+33]; tile t reads block t//8.
            TB = []
            for b in range(4):
                tb = singles.tile([128, 34, 130], F32, name=f"TB{b}")
                nc.vector.memset(tb[0:64, :, 0:1], 0.0)
                nc.vector.memset(tb[0:64, :, 129:130], 0.0)
                nc.vector.memset(tb[64:128, :, 128:130], 0.0)
                if b == 0:
                    nc.vector.memset(tb[:, 0, :], 0.0)
                if b == 3:
                    nc.vector.memset(tb[:, 33, :], 0.0)
                g0 = 32 * b          # first padded row in block
                lo = max(g0, 1)      # first interior padded row
                hi = min(g0 + 33, 128)  # last interior padded row
                # lower: P[c, r, 1+x] = img[c, r-1, x]
                nc.sync.dma_start(
                    out=tb[0:64, lo - g0 : hi - g0 + 1, 1:129],
                    in_=img_d[:, lo - 1 : hi, :],
                )
                # upper: U[c, r, x] = P[c, r, x+1] = img[c, r-1, x]
                nc.sync.dma_start(
                    out=tb[64:128, lo - g0 : hi - g0 + 1, 0:128],
                    in_=img_d[:, lo - 1 : hi, :],
                )
                TB.append(tb)

            # ---- depth rows, partition-shifted copies D_i[y, X] = D_pad[y+i, X] ----
            D0 = singles.tile([128, 130], F32)
            D1 = singles.tile([128, 130], F32)
            D2 = singles.tile([128, 130], F32)
            nc.vector.memset(D0[:, :], 0.0)
            nc.vector.memset(D1[:, :], 0.0)
            nc.vector.memset(D2[:, :], 0.0)
            nc.sync.dma_start(out=D0[1:128, 1:129], in_=dep_d[0:127, :])
            nc.sync.dma_start(out=D1[0:128, 1:129], in_=dep_d[:, :])
            nc.sync.dma_start(out=D2[0:127, 1:129], in_=dep_d[1:128, :])
            Ds = [D0, D1, D2]

            # ---- weights / selection matrices / bias ----
            wp_sb = singles.tile([128, 4, 64], F32R)
            ws8_sb = singles.tile([64, 64], F32R)
            sp_sb = singles.tile([9, 4, 128], F32R)
            ss8_sb = singles.tile([9, 64], F32R)
            bias_sb = singles.tile([64, 1], F32)
            nc.sync.dma_start(out=wp_sb[:], in_=wp_d[:].bitcast(F32R))
            nc.sync.dma_start(out=ws8_sb[:], in_=ws8_d[:].bitcast(F32R))
            nc.sync.dma_start(out=sp_sb[:], in_=sp_d[:].bitcast(F32R))
            nc.sync.dma_start(out=ss8_sb[:], in_=ss8_d[:].bitcast(F32R))
            nc.sync.dma_start(out=bias_sb[:], in_=bias_d[:])

            # ---- sim in [y, x] layout: sim[k][y, x] = exp(-a*|D_i[y, x+j] - D0[y, x]|) ----
            sim_yx = singles.tile([128, 9, 128], F32)
            for k in range(9):
                i, j = k // 3, k % 3
                nc.vector.tensor_tensor(
                    out=sim_yx[:, k, :],
                    in0=Ds[i][:, j : j + 128],
                    in1=D0[:, 0:128],
                    op=mybir.AluOpType.subtract,
                )
            nc.scalar.activation(
                out=sim_yx[:, :, :],
                in_=sim_yx[:, :, :],
                func=mybir.ActivationFunctionType.Abs,
                scale=ALPHA,
            )
            nc.scalar.activation(
                out=sim_yx[:, :, :],
                in_=sim_yx[:, :, :],
                func=mybir.ActivationFunctionType.Exp,
                scale=-1.0,
            )
            # flatten [y, k, x] -> [k, (y x)] through DRAM
            sim_dram = drampool.tile([128, 9, 128], F32)
            nc.sync.dma_start(out=sim_dram[:], in_=sim_yx[:])
            sim_flat = singles.tile([9, 128, 128], F32R)
            nc.sync.dma_start(
                out=sim_flat[:, :, :],
                in_=sim_dram.rearrange("y k x -> k y x").bitcast(F32R),
            )

            # ---- main loop: 32 tiles of 4 image rows (512 px each) ----
            import contextlib

            no_pe = probe == "nope"
            if no_pe:
                simconst = singles.tile([128, 4, 128], F32)
                nc.vector.memset(simconst[:], 0.5)

            loop_ctx = (
                tc.For_i(0, loop_reps, 1) if loop_reps else contextlib.nullcontext()
            )
            with loop_ctx:
              for t in range(32):
                  y0 = 4 * t
                  T1 = TB[t // 8]
                  yl = y0 - 32 * (t // 8)  # local row of this tile in its block
                  sfw = sim_flat[:, y0 : y0 + 4, :]  # [9, 4, 128]

                  group_mods = []
                  # wide groups: 3 pairs + merged single (2,5)
                  for g in range(4):
                      rep = reps.tile([128, 4, 128], F32, tag="rep", name=f"rep_{t}_{g}")
                      nc.tensor.matmul(
                          out=rep[:],
                          lhsT=sp_sb[:, g, :],
                          rhs=sfw,
                          start=True,
                          stop=True,
                      )
                      mod = mods.tile([128, 4, 128], F32R, tag="modp", name=f"mod_{t}_{g}")
                      if g == 0:
                          # GPSIMD path: ACT copies the sim-replica PSUM->SBUF
                          # (GPSIMD has no PSUM access), GPSIMD multiplies.
                          rep_sb = repsb.tile(
                              [128, 4, 128], F32, tag="repsb_p", name=f"repsb_{t}"
                          )
                          nc.scalar.activation(
                              out=rep_sb[:],
                              in_=rep[:],
                              func=mybir.ActivationFunctionType.Copy,
                          )
                          nc.gpsimd.tensor_tensor(
                              out=mod[:],
                              in0=T1[:, yl + g : yl + g + 4, 0:128],
                              in1=rep_sb[:],
                              op=mybir.AluOpType.mult,
                          )
                      elif g < 3:
                          nc.vector.tensor_tensor(
                              out=mod[:],
                              in0=T1[:, yl + g : yl + g + 4, 0:128],
                              in1=rep[:],
                              op=mybir.AluOpType.mult,
                          )
                      else:
                          # k=2 from the lower copy
                          nc.vector.tensor_tensor(
                              out=mod[0:64],
                              in0=T1[0:64, yl : yl + 4, 2:130],
                              in1=rep[0:64],
                              op=mybir.AluOpType.mult,
                          )
                          # k=5 via the upper (+1 shifted) copy
                          nc.vector.tensor_tensor(
                              out=mod[64:128],
                              in0=T1[64:128, yl + 1 : yl + 5, 1:129],
                              in1=rep[64:128],
                              op=mybir.AluOpType.mult,
                          )
                      group_mods.append(mod)

                  # narrow group: offset 8 on GPSIMD
                  rep8 = reps.tile([64, 4, 128], F32, tag="rep", name=f"rep8_{t}")
                  nc.tensor.matmul(
                      out=rep8[:], lhsT=ss8_sb[:, :], rhs=sfw, start=True, stop=True
                  )
                  mod8 = mods.tile([64, 4, 128], F32R, tag="mods", name=f"mod8_{t}")
                  rep8_sb = repsb.tile(
                      [64, 4, 128], F32, tag="repsb_s", name=f"rep8sb_{t}"
                  )
                  nc.scalar.activation(
                      out=rep8_sb[:],
                      in_=rep8[:],
                      func=mybir.ActivationFunctionType.Copy,
                  )
                  nc.gpsimd.tensor_tensor(
                      out=mod8[:],
                      in0=T1[0:64, yl + 2 : yl + 6, 2:130],
                      in1=rep8_sb[:],
                      op=mybir.AluOpType.mult,
                  )

                  out_ps = outps.tile([64, 4, 128], F32, tag="outps", name=f"outps_{t}")
                  for g in range(4):
                      nc.tensor.matmul(
                          out=out_ps[:],
                          lhsT=wp_sb[:, g, :],
                          rhs=group_mods[g][:],
                          start=(g == 0),
                          stop=False,
                      )
                  nc.tensor.matmul(
                      out=out_ps[:], lhsT=ws8_sb[:, :], rhs=mod8[:], start=False, stop=True
                  )

                  out_sb = outsb.tile([64, 4, 128], F32, tag="outsb", name=f"outsb_{t}")
                  nc.scalar.activation(
                      out=out_sb[:],
                      in_=out_ps[:],
                      func=mybir.ActivationFunctionType.Identity,
                      bias=bias_sb[:, 0:1],
                      scale=1.0,
                  )
                  nc.sync.dma_start(out=out_d[:, y0 : y0 + 4, :], in_=out_sb[:])

    _split_excess_waits(nc)
    return nc


_NC_CACHE = None


def _get_nc():
    global _NC_CACHE
    if _NC_CACHE is None:
        _NC_CACHE = build_bass()
    return _NC_CACHE


def _prep_operands(weight, bias):
    wp = np.zeros((128, 4, 64), np.float32)
    for p in range(3):
        wp[:64, p, :] = weight[:, :, p, 0].T
        wp[64:, p, :] = weight[:, :, p, 1].T
    wp[:64, 3, :] = weight[:, :, 0, 2].T  # k=2
    wp[64:, 3, :] = weight[:, :, 1, 2].T  # k=5
    ws8 = np.ascontiguousarray(weight[:, :, 2, 2].T)  # k=8
    sp = np.zeros((9, 4, 128), np.float32)
    for p in range(3):
        sp[3 * p + 0, p, 0:64] = 1.0
        sp[3 * p + 1, p, 64:128] = 1.0
    sp[2, 3, 0:64] = 1.0
    sp[5, 3, 64:128] = 1.0
    ss8 = np.zeros((9, 64), np.float32)
    ss8[8, :] = 1.0
    bias2 = np.ascontiguousarray(bias.reshape(64, 1))
    return wp, ws8, sp, ss8, bias2


def kernel(image, depth, weight, bias, **kwargs):
    image = np.ascontiguousarray(np.asarray(image, dtype=np.float32))
    depth = np.ascontiguousarray(np.asarray(depth, dtype=np.float32))
    weight = np.ascontiguousarray(np.asarray(weight, dtype=np.float32))
    bias = np.ascontiguousarray(np.asarray(bias, dtype=np.float32))

    B = image.shape[0]
    assert B == N_CORES, f"expected batch {N_CORES}, got {B}"

    wp, ws8, sp, ss8, bias2 = _prep_operands(weight, bias)

    global _last_in_maps
    nc = _get_nc()
    in_maps = [
        {
            "image": image[b],
            "depth": depth[b, 0],
            "wp": wp,
            "ws8": ws8,
            "sp": sp,
            "ss8": ss8,
            "bias": bias2,
        }
        for b in range(B)
    ]
    _last_in_maps = in_maps
    res = run_bass_kernel_spmd(nc, in_maps, core_ids=list(range(N_CORES)))
    out = np.stack([r["out"] for r in res.results], axis=0)
    return out.astype(np.float32)

